# revision 10
# baseline (speedup 1.0000x reference)
"""Trainium2 Bass kernel for nn_EquivarianceNetwork (grouped 4-layer MLP).

Math (per sample b, TWO_N=16 groups, D=64):
  xr = x.reshape(B, 16, 64)
  scalars[b, n, m] = <xr[b,n], xr[b,m]>                  # [B, 256]
  per group l: h = tanh(...W0/W1/W2...), coeffs = h @ W3 + b3   # [B, 16]
  out[b, l*64:(l+1)*64] = sum_n coeffs[l,b,n] * xr[b,n]

Distribution: data-parallel over batch across 8 cores (weights replicated).
Per core B_local = 2048.

Engine plan per core:
  - PE: all GEMMs in float32r (fp32 data, ~TF32 matmul precision, 1 cyc/row
    at N=512) with feature-major activations; small transposes.
  - ACT: tanh+bias (PSUM->SBUF, float32r out), L3 bias add, PSUM->SBUF copies.
  - DVE: Gram reduces + 2 mult-deltas + mirrors, final-stage reduces and
    half the final mults.
  - GPSIMD: remaining Gram mult-deltas, other half of final mults.
  - Weights streamed from HBM per group l, double buffered; biases preloaded.
"""
import numpy as np

import concourse.bass as bass
import concourse.mybir as mybir
import concourse.tile as tile
from concourse import bacc
from concourse.bass_utils import run_bass_kernel_spmd
from concourse.masks import make_identity

F32 = mybir.dt.float32
F32R = mybir.dt.float32r
TANH = mybir.ActivationFunctionType.Tanh

N_CORES = 8
B = 16384
TWO_N = 16
D = 64
B_LOC = B // N_CORES          # 2048
N_SUB = B_LOC // 128          # 16 subtiles of 128 samples
N_BT = B_LOC // 512           # 4 batch tiles of 512 (matmul free dim)
H = 1024                      # hidden width
K_IN = 256                    # 16*16 scalars


def _build_program():
    nc = bacc.Bacc()

    x = nc.declare_dram_parameter("x", [B_LOC, TWO_N * D], F32, isOutput=False)
    W0 = nc.declare_dram_parameter("W0", [TWO_N, K_IN, H], F32R, isOutput=False)
    W1 = nc.declare_dram_parameter("W1", [TWO_N, H, H], F32R, isOutput=False)
    W2 = nc.declare_dram_parameter("W2", [TWO_N, H, H], F32R, isOutput=False)
    W3 = nc.declare_dram_parameter("W3", [TWO_N, H, TWO_N], F32R, isOutput=False)
    b0 = nc.declare_dram_parameter("b0", [TWO_N, H], F32, isOutput=False)
    b1 = nc.declare_dram_parameter("b1", [TWO_N, H], F32, isOutput=False)
    b2 = nc.declare_dram_parameter("b2", [TWO_N, H], F32, isOutput=False)
    b3 = nc.declare_dram_parameter("b3", [TWO_N, TWO_N], F32, isOutput=False)
    y = nc.declare_dram_parameter("y", [B_LOC, TWO_N * D], F32, isOutput=True)

    with tile.TileContext(nc) as tc:
        with tc.tile_pool(name="res", bufs=1) as res, \
             tc.tile_pool(name="xg", bufs=4) as xgp, \
             tc.tile_pool(name="work", bufs=2) as wk, \
             tc.tile_pool(name="w0p", bufs=2) as w0p, \
             tc.tile_pool(name="w12p", bufs=5) as w12p, \
             tc.tile_pool(name="w3p", bufs=2) as w3p, \
             tc.tile_pool(name="hp", bufs=2) as hp, \
             tc.tile_pool(name="fin", bufs=4) as finp, \
             tc.tile_pool(name="ps", bufs=4, space="PSUM") as ps:

            ident = res.tile([128, 128], F32)
            make_identity(nc, ident)

            # ---- biases: preload all groups once, transposed on PE ----
            # b012_all[p, li, ot, l] = b_li[l, ot*128 + p]
            b012_all = res.tile([128, 3, 8, TWO_N], F32)
            b3_all = res.tile([16, TWO_N], F32)   # [n, l]
            for li, bsrc in enumerate((b0, b1, b2)):
                bnat = wk.tile([TWO_N, H], F32, name=f"bnat{li}", tag="bnat")
                nc.sync.dma_start(out=bnat, in_=bsrc[:, :])
                for ot in range(8):
                    pt = ps.tile([128, 128], F32, name="tpb", tag="tp", bufs=2)
                    nc.tensor.transpose(
                        pt[:, 0:TWO_N], bnat[:, 128 * ot:128 * (ot + 1)],
                        ident[0:TWO_N, 0:TWO_N])
                    nc.scalar.copy(b012_all[:, li, ot, :], pt[:, 0:TWO_N])
            b3nat = wk.tile([TWO_N, TWO_N], F32, name="b3nat", tag="bnat")
            nc.sync.dma_start(out=b3nat, in_=b3[:, :])
            pt = ps.tile([128, 128], F32, name="tpb3", tag="tp", bufs=2)
            nc.tensor.transpose(pt[0:TWO_N, 0:TWO_N], b3nat[:, :],
                                ident[0:TWO_N, 0:TWO_N])
            nc.scalar.copy(b3_all[:, :], pt[0:TWO_N, 0:TWO_N])

            # resident: transposed scalars [256, B_LOC] as 2 partition tiles
            scalT = [res.tile([128, B_LOC], F32R, name=f"scalT{i}")
                     for i in range(2)]
            # resident: coeffs batch-major per subtile [128, 256] (col l*16+n)
            coeff = [res.tile([128, 256], F32, name=f"coeff{s}")
                     for s in range(N_SUB)]

            # ---------------- Gram for one subtile of 128 samples ----------
            def gram(s):
                xg = xgp.tile([128, TWO_N * D], F32, name="xg", tag="xg")
                nc.sync.dma_start(out=xg, in_=x[128 * s:128 * (s + 1), :])
                sbm = wk.tile([128, K_IN], F32, name="sbm", tag="sbm")
                prod = wk.tile([128, TWO_N * D], F32, name="prod", tag="prod")
                if s < 2:
                    # first use of each sbm slot: zero the mirror columns so
                    # the (m>n) garbage cols are finite (W0 is host-folded
                    # into the upper triangle; lower-triangle weights are 0).
                    nc.gpsimd.memset(sbm[:, :], 0.0)
                for dl in range(TWO_N):
                    npair = TWO_N - dl
                    meng = nc.vector if dl < 3 else nc.gpsimd
                    meng.tensor_mul(
                        prod[:, 0:npair * D],
                        xg[:, 0:npair * D],
                        xg[:, dl * D:(dl + npair) * D],
                    )
                    dst = bass.AP(tensor=sbm.tensor, offset=sbm.offset + dl,
                                  ap=[sbm.ap[0], [17, npair]])
                    nc.vector.tensor_reduce(
                        dst, prod[:, 0:npair * D].rearrange(
                            "p (n d) -> p n d", d=D),
                        axis=mybir.AxisListType.X, op=mybir.AluOpType.add)
                for i in range(2):
                    pt = ps.tile([128, 128], F32, name="tp", tag="tp", bufs=2)
                    nc.tensor.transpose(
                        pt[:, :], sbm[:, 128 * i:128 * (i + 1)], ident)
                    nc.scalar.copy(
                        scalT[i][:, 128 * s:128 * (s + 1)], pt[:, :])

            # ---- final contraction for one (l, subtile):
            # y[bsub, l*64+d] = sum_n coeff[b, 16l+n] * x[b, 64n+d]
            def final_unit(l, s):
                xg = xgp.tile([128, TWO_N * D], F32, name="xg2", tag="xg")
                nc.sync.dma_start(out=xg, in_=x[128 * s:128 * (s + 1), :])
                prod = wk.tile([128, TWO_N * D], F32, name="prod2", tag="prod")
                c = coeff[s]
                in1 = bass.AP(tensor=c.tensor, offset=c.offset + 16 * l,
                              ap=[c.ap[0], [1, TWO_N], [0, D]])
                meng = nc.vector if s % 4 != 3 else nc.gpsimd
                meng.tensor_mul(
                    prod[:, :].rearrange("p (n d) -> p n d", d=D),
                    xg[:, :].rearrange("p (n d) -> p n d", d=D),
                    in1)
                meng.tensor_add(prod[:, 0:512], prod[:, 0:512], prod[:, 512:1024])
                meng.tensor_add(prod[:, 0:256], prod[:, 0:256], prod[:, 256:512])
                meng.tensor_add(prod[:, 0:128], prod[:, 0:128], prod[:, 128:256])
                fcol = finp.tile([128, D], F32, name="fcol", tag="fcol")
                meng.tensor_add(fcol[:, :], prod[:, 0:D], prod[:, D:2 * D])
                nc.sync.dma_start(
                    out=y[128 * s:128 * (s + 1), D * l:D * (l + 1)],
                    in_=fcol[:, :])

            # ---------------- Phase B: grouped MLP ----------------
            # The first Gram group is hoisted ahead of the l=0 weight
            # stream; inside l==0, group k+1 is emitted after MLP bt k so
            # the PE stream never waits on a group it doesn't need yet.
            for s in range(4):
                gram(s)

            for l in range(TWO_N):
                w0t = w0p.tile([128, 2, H], F32R, name="w0t", tag="w0")
                nc.sync.dma_start(
                    out=w0t,
                    in_=W0[l, :, :].rearrange("(t p) m -> p t m", p=128))
                w1h = []
                w2h = []
                for hname, Wsrc, lst in (("w1", W1, w1h), ("w2", W2, w2h)):
                    for half in range(2):
                        wt = w12p.tile([128, 4, H], F32R,
                                       name=f"{hname}{half}", tag="w12")
                        nc.sync.dma_start(
                            out=wt,
                            in_=Wsrc[l, 512 * half:512 * (half + 1), :]
                            .rearrange("(t p) m -> p t m", p=128))
                        lst.append(wt)
                w3t = w3p.tile([128, 8, TWO_N], F32R, name="w3t", tag="w3")
                nc.sync.dma_start(
                    out=w3t,
                    in_=W3[l, :, :].rearrange("(t p) m -> p t m", p=128))

                for bt in range(N_BT):
                    bs = 512 * bt
                    # L0: scalT -> h0
                    h0 = hp.tile([128, 8, 512], F32R, name="h0", tag="h")
                    for ot in range(8):
                        pt = ps.tile([128, 512], F32, name="mlp", tag="mlp",
                                     bufs=4)
                        for kt in range(2):
                            nc.tensor.matmul(
                                pt[:, :],
                                w0t[:, kt, 128 * ot:128 * (ot + 1)],
                                scalT[kt][:, bs:bs + 512],
                                start=(kt == 0), stop=(kt == 1))
                        nc.scalar.activation(
                            h0[:, ot, :], pt[:, :], TANH,
                            bias=b012_all[:, 0, ot, l:l + 1])
                    # L1, L2
                    hin = h0
                    for li, whalves in ((1, w1h), (2, w2h)):
                        hout = hp.tile([128, 8, 512], F32R,
                                       name=f"h{li}", tag="h")
                        for ot in range(8):
                            pt = ps.tile([128, 512], F32, name="mlp",
                                         tag="mlp", bufs=4)
                            for kt in range(8):
                                nc.tensor.matmul(
                                    pt[:, :],
                                    whalves[kt // 4][:, kt % 4,
                                                     128 * ot:128 * (ot + 1)],
                                    hin[:, kt, :],
                                    start=(kt == 0), stop=(kt == 7))
                            nc.scalar.activation(
                                hout[:, ot, :], pt[:, :], TANH,
                                bias=b012_all[:, li, ot, l:l + 1])
                        hin = hout
                    # L3 -> coeffs [16, 512] + bias, transpose to batch-major
                    p3 = ps.tile([16, 512], F32, name="p3", tag="p3", bufs=2)
                    for kt in range(8):
                        nc.tensor.matmul(p3[:, :], w3t[:, kt, :],
                                         hin[:, kt, :],
                                         start=(kt == 0), stop=(kt == 7))
                    csb = wk.tile([16, 512], F32, name="csb", tag="csb")
                    nc.scalar.add(csb[:, :], p3[:, :], b3_all[:, l:l + 1])
                    for j in range(4):
                        tp = ps.tile([128, 16], F32, name="tp2", tag="tp",
                                     bufs=2)
                        nc.tensor.transpose(
                            tp[:, 0:16], csb[:, 128 * j:128 * (j + 1)],
                            ident[0:16, 0:16])
                        sub = 4 * bt + j
                        nc.scalar.copy(
                            coeff[sub][:, 16 * l:16 * (l + 1)], tp[:, 0:16])

                    # finals for this bt's subtiles (their coeff cols are
                    # ready); spreads DVE work and xg DMA over the window
                    for s in range(4 * bt, 4 * bt + 4):
                        final_unit(l, s)

                    if l == 0 and bt < 3:
                        for s in range(4 * bt + 4, 4 * bt + 8):
                            gram(s)

    nc.finalize()
    return nc


_NC = None


def kernel(x, W0, b0, W1, b1, W2, b2, W3, b3):
    global _NC
    if _NC is None:
        _NC = _build_program()

    x = np.ascontiguousarray(np.asarray(x, dtype=np.float32))
    # Fold W0 over the symmetric scalar pairs: scalars[b,(n,m)] == [b,(m,n)],
    # and the kernel only materializes the upper triangle (col 16n+m, n<=m).
    # h0 = scal @ W0 is preserved exactly by moving the lower-triangle
    # weights onto their mirrored counterpart and zeroing them.
    W0f = np.asarray(W0, np.float32).reshape(TWO_N, TWO_N, TWO_N, H).copy()
    for n in range(TWO_N):
        for m in range(n + 1, TWO_N):
            W0f[:, n, m, :] += W0f[:, m, n, :]
            W0f[:, m, n, :] = 0.0
    W0f = W0f.reshape(TWO_N, K_IN, H)
    shared = {
        "W0": np.ascontiguousarray(W0f),
        "W1": np.ascontiguousarray(np.asarray(W1, np.float32)),
        "W2": np.ascontiguousarray(np.asarray(W2, np.float32)),
        "W3": np.ascontiguousarray(np.asarray(W3, np.float32)),
        "b0": np.ascontiguousarray(np.asarray(b0, np.float32)),
        "b1": np.ascontiguousarray(np.asarray(b1, np.float32)),
        "b2": np.ascontiguousarray(np.asarray(b2, np.float32)),
        "b3": np.ascontiguousarray(np.asarray(b3, np.float32)),
    }
    in_maps = []
    for c in range(N_CORES):
        m = dict(shared)
        m["x"] = x[B_LOC * c:B_LOC * (c + 1), :]
        in_maps.append(m)
    res = run_bass_kernel_spmd(_NC, in_maps, list(range(N_CORES)))
    return np.concatenate([res.results[c]["y"] for c in range(N_CORES)],
                          axis=0)


# revision 11
# speedup vs baseline: 1.0544x; 1.0544x over previous
"""Trainium2 Bass kernel for nn_EquivarianceNetwork (grouped 4-layer MLP).

Math (per sample b, TWO_N=16 groups, D=64):
  xr = x.reshape(B, 16, 64)
  scalars[b, n, m] = <xr[b,n], xr[b,m]>                  # [B, 256]
  per group l: h = tanh(...W0/W1/W2...), coeffs = h @ W3 + b3   # [B, 16]
  out[b, l*64:(l+1)*64] = sum_n coeffs[l,b,n] * xr[b,n]

Distribution: data-parallel over batch across 8 cores (weights replicated).
Per core B_local = 2048.

Engine plan per core:
  - PE: all GEMMs in float32r (fp32 data, ~TF32 matmul precision, 1 cyc/row
    at N=512) with feature-major activations; small transposes.
  - ACT: tanh+bias (PSUM->SBUF, float32r out), L3 bias add, PSUM->SBUF copies.
  - DVE: Gram reduces + 2 mult-deltas + mirrors, final-stage reduces and
    half the final mults.
  - GPSIMD: remaining Gram mult-deltas, other half of final mults.
  - Weights streamed from HBM per group l, double buffered; biases preloaded.
"""
import numpy as np

import concourse.bass as bass
import concourse.mybir as mybir
import concourse.tile as tile
from concourse import bacc
from concourse.bass_utils import run_bass_kernel_spmd
from concourse.masks import make_identity

F32 = mybir.dt.float32
F32R = mybir.dt.float32r
TANH = mybir.ActivationFunctionType.Tanh

N_CORES = 8
B = 16384
TWO_N = 16
D = 64
B_LOC = B // N_CORES          # 2048
N_SUB = B_LOC // 128          # 16 subtiles of 128 samples
N_BT = B_LOC // 512           # 4 batch tiles of 512 (matmul free dim)
H = 1024                      # hidden width
K_IN = 256                    # 16*16 scalars


def _build_program():
    nc = bacc.Bacc()

    x = nc.declare_dram_parameter("x", [B_LOC, TWO_N * D], F32, isOutput=False)
    W0 = nc.declare_dram_parameter("W0", [TWO_N, K_IN, H], F32R, isOutput=False)
    W1 = nc.declare_dram_parameter("W1", [TWO_N, H, H], F32R, isOutput=False)
    W2 = nc.declare_dram_parameter("W2", [TWO_N, H, H], F32R, isOutput=False)
    W3 = nc.declare_dram_parameter("W3", [TWO_N, H, TWO_N], F32R, isOutput=False)
    b0 = nc.declare_dram_parameter("b0", [TWO_N, H], F32, isOutput=False)
    b1 = nc.declare_dram_parameter("b1", [TWO_N, H], F32, isOutput=False)
    b2 = nc.declare_dram_parameter("b2", [TWO_N, H], F32, isOutput=False)
    b3 = nc.declare_dram_parameter("b3", [TWO_N, TWO_N], F32, isOutput=False)
    y = nc.declare_dram_parameter("y", [B_LOC, TWO_N * D], F32, isOutput=True)

    with tile.TileContext(nc) as tc:
        with tc.tile_pool(name="res", bufs=1) as res, \
             tc.tile_pool(name="xg", bufs=4) as xgp, \
             tc.tile_pool(name="work", bufs=2) as wk, \
             tc.tile_pool(name="w0p", bufs=2) as w0p, \
             tc.tile_pool(name="w12p", bufs=5) as w12p, \
             tc.tile_pool(name="w3p", bufs=2) as w3p, \
             tc.tile_pool(name="hp", bufs=2) as hp, \
             tc.tile_pool(name="fin", bufs=4) as finp, \
             tc.tile_pool(name="ps", bufs=4, space="PSUM") as ps:

            ident = res.tile([128, 128], F32)
            make_identity(nc, ident)

            # ---- biases: preload all groups once, transposed on PE ----
            # b012_all[p, li, ot, l] = b_li[l, ot*128 + p]
            b012_all = res.tile([128, 3, 8, TWO_N], F32)
            b3_all = res.tile([16, TWO_N], F32)   # [n, l]
            for li, bsrc in enumerate((b0, b1, b2)):
                bnat = wk.tile([TWO_N, H], F32, name=f"bnat{li}", tag="bnat")
                nc.sync.dma_start(out=bnat, in_=bsrc[:, :])
                for ot in range(8):
                    pt = ps.tile([128, 128], F32, name="tpb", tag="tp", bufs=2)
                    nc.tensor.transpose(
                        pt[:, 0:TWO_N], bnat[:, 128 * ot:128 * (ot + 1)],
                        ident[0:TWO_N, 0:TWO_N])
                    nc.scalar.copy(b012_all[:, li, ot, :], pt[:, 0:TWO_N])
            b3nat = wk.tile([TWO_N, TWO_N], F32, name="b3nat", tag="bnat")
            nc.sync.dma_start(out=b3nat, in_=b3[:, :])
            pt = ps.tile([128, 128], F32, name="tpb3", tag="tp", bufs=2)
            nc.tensor.transpose(pt[0:TWO_N, 0:TWO_N], b3nat[:, :],
                                ident[0:TWO_N, 0:TWO_N])
            nc.scalar.copy(b3_all[:, :], pt[0:TWO_N, 0:TWO_N])

            # resident: transposed scalars [256, B_LOC] as 2 partition tiles
            scalT = [res.tile([128, B_LOC], F32R, name=f"scalT{i}")
                     for i in range(2)]
            # resident: coeffs batch-major per subtile [128, 256] (col l*16+n)
            coeff = [res.tile([128, 256], F32, name=f"coeff{s}")
                     for s in range(N_SUB)]

            # ---------------- Gram for one subtile of 128 samples ----------
            def gram(s):
                xg = xgp.tile([128, TWO_N * D], F32, name="xg", tag="xg")
                nc.sync.dma_start(out=xg, in_=x[128 * s:128 * (s + 1), :])
                sbm = wk.tile([128, K_IN], F32, name="sbm", tag="sbm")
                prod = wk.tile([128, TWO_N * D], F32, name="prod", tag="prod")
                if s < 2:
                    # first use of each sbm slot: zero the mirror columns so
                    # the (m>n) garbage cols are finite (W0 is host-folded
                    # into the upper triangle; lower-triangle weights are 0).
                    nc.gpsimd.memset(sbm[:, :], 0.0)
                for dl in range(TWO_N):
                    npair = TWO_N - dl
                    meng = nc.vector if dl < 3 else nc.gpsimd
                    meng.tensor_mul(
                        prod[:, 0:npair * D],
                        xg[:, 0:npair * D],
                        xg[:, dl * D:(dl + npair) * D],
                    )
                    dst = bass.AP(tensor=sbm.tensor, offset=sbm.offset + dl,
                                  ap=[sbm.ap[0], [17, npair]])
                    nc.vector.tensor_reduce(
                        dst, prod[:, 0:npair * D].rearrange(
                            "p (n d) -> p n d", d=D),
                        axis=mybir.AxisListType.X, op=mybir.AluOpType.add)
                for i in range(2):
                    pt = ps.tile([128, 128], F32, name="tp", tag="tp", bufs=2)
                    nc.tensor.transpose(
                        pt[:, :], sbm[:, 128 * i:128 * (i + 1)], ident)
                    nc.scalar.copy(
                        scalT[i][:, 128 * s:128 * (s + 1)], pt[:, :])

            # ---- final contraction for one (l, subtile):
            # y[bsub, l*64+d] = sum_n coeff[b, 16l+n] * x[b, 64n+d]
            def final_unit(l, s):
                xg = xgp.tile([128, TWO_N * D], F32, name="xg2", tag="xg")
                nc.sync.dma_start(out=xg, in_=x[128 * s:128 * (s + 1), :])
                prod = wk.tile([128, TWO_N * D], F32, name="prod2", tag="prod")
                c = coeff[s]
                in1 = bass.AP(tensor=c.tensor, offset=c.offset + 16 * l,
                              ap=[c.ap[0], [1, TWO_N], [0, D]])
                meng = nc.vector if s % 4 != 3 else nc.gpsimd
                meng.tensor_mul(
                    prod[:, :].rearrange("p (n d) -> p n d", d=D),
                    xg[:, :].rearrange("p (n d) -> p n d", d=D),
                    in1)
                meng.tensor_add(prod[:, 0:512], prod[:, 0:512], prod[:, 512:1024])
                meng.tensor_add(prod[:, 0:256], prod[:, 0:256], prod[:, 256:512])
                meng.tensor_add(prod[:, 0:128], prod[:, 0:128], prod[:, 128:256])
                fcol = finp.tile([128, D], F32, name="fcol", tag="fcol")
                meng.tensor_add(fcol[:, :], prod[:, 0:D], prod[:, D:2 * D])
                nc.sync.dma_start(
                    out=y[128 * s:128 * (s + 1), D * l:D * (l + 1)],
                    in_=fcol[:, :])

            # ---------------- Phase B: grouped MLP ----------------
            # The first Gram group is hoisted ahead of the l=0 weight
            # stream; inside l==0, group k+1 is emitted after MLP bt k so
            # the PE stream never waits on a group it doesn't need yet.
            for s in range(4):
                gram(s)

            for l in range(TWO_N):
                w0t = w0p.tile([128, 2, H], F32R, name="w0t", tag="w0")
                nc.sync.dma_start(
                    out=w0t,
                    in_=W0[l, :, :].rearrange("(t p) m -> p t m", p=128))
                w1h = []
                w2h = []
                for hname, Wsrc, lst in (("w1", W1, w1h), ("w2", W2, w2h)):
                    for half in range(2):
                        wt = w12p.tile([128, 4, H], F32R,
                                       name=f"{hname}{half}", tag="w12")
                        nc.sync.dma_start(
                            out=wt,
                            in_=Wsrc[l, 512 * half:512 * (half + 1), :]
                            .rearrange("(t p) m -> p t m", p=128))
                        lst.append(wt)
                w3t = w3p.tile([128, 8, TWO_N], F32R, name="w3t", tag="w3")
                nc.sync.dma_start(
                    out=w3t,
                    in_=W3[l, :, :].rearrange("(t p) m -> p t m", p=128))

                for bt in range(N_BT):
                    bs = 512 * bt
                    # L0: scalT -> h0
                    h0 = hp.tile([128, 8, 512], F32R, name="h0", tag="h")
                    for ot in range(8):
                        pt = ps.tile([128, 512], F32, name="mlp", tag="mlp",
                                     bufs=4)
                        for kt in range(2):
                            nc.tensor.matmul(
                                pt[:, :],
                                w0t[:, kt, 128 * ot:128 * (ot + 1)],
                                scalT[kt][:, bs:bs + 512],
                                start=(kt == 0), stop=(kt == 1))
                        nc.scalar.activation(
                            h0[:, ot, :], pt[:, :], TANH,
                            bias=b012_all[:, 0, ot, l:l + 1])
                    # L1, L2
                    hin = h0
                    for li, whalves in ((1, w1h), (2, w2h)):
                        hout = hp.tile([128, 8, 512], F32R,
                                       name=f"h{li}", tag="h")
                        for ot in range(8):
                            pt = ps.tile([128, 512], F32, name="mlp",
                                         tag="mlp", bufs=4)
                            for kt in range(8):
                                nc.tensor.matmul(
                                    pt[:, :],
                                    whalves[kt // 4][:, kt % 4,
                                                     128 * ot:128 * (ot + 1)],
                                    hin[:, kt, :],
                                    start=(kt == 0), stop=(kt == 7))
                            nc.scalar.activation(
                                hout[:, ot, :], pt[:, :], TANH,
                                bias=b012_all[:, li, ot, l:l + 1])
                        hin = hout
                    # L3 -> coeffs [16, 512] + bias, transpose to batch-major
                    p3 = ps.tile([16, 512], F32, name="p3", tag="p3", bufs=2)
                    for kt in range(8):
                        nc.tensor.matmul(p3[:, :], w3t[:, kt, :],
                                         hin[:, kt, :],
                                         start=(kt == 0), stop=(kt == 7))
                    csb = wk.tile([16, 512], F32, name="csb", tag="csb")
                    nc.scalar.add(csb[:, :], p3[:, :], b3_all[:, l:l + 1])
                    for j in range(4):
                        tp = ps.tile([128, 16], F32, name="tp2", tag="tp",
                                     bufs=2)
                        nc.tensor.transpose(
                            tp[:, 0:16], csb[:, 128 * j:128 * (j + 1)],
                            ident[0:16, 0:16])
                        sub = 4 * bt + j
                        nc.scalar.copy(
                            coeff[sub][:, 16 * l:16 * (l + 1)], tp[:, 0:16])

                    if l == 0:
                        # l=0 is Gram-bound: emit the next Gram group here
                        # and defer finals to the end of the group loop.
                        if bt < 3:
                            for s in range(4 * bt + 4, 4 * bt + 8):
                                gram(s)
                    else:
                        # finals for this bt's subtiles (their coeff cols
                        # are ready); spreads DVE work and the xg DMAs
                        for s in range(4 * bt, 4 * bt + 4):
                            final_unit(l, s)

                if l == 0:
                    for s in range(N_SUB):
                        final_unit(l, s)

    nc.finalize()
    return nc


_NC = None


def kernel(x, W0, b0, W1, b1, W2, b2, W3, b3):
    global _NC
    if _NC is None:
        _NC = _build_program()

    x = np.ascontiguousarray(np.asarray(x, dtype=np.float32))
    # Fold W0 over the symmetric scalar pairs: scalars[b,(n,m)] == [b,(m,n)],
    # and the kernel only materializes the upper triangle (col 16n+m, n<=m).
    # h0 = scal @ W0 is preserved exactly by moving the lower-triangle
    # weights onto their mirrored counterpart and zeroing them.
    W0f = np.asarray(W0, np.float32).reshape(TWO_N, TWO_N, TWO_N, H).copy()
    for n in range(TWO_N):
        for m in range(n + 1, TWO_N):
            W0f[:, n, m, :] += W0f[:, m, n, :]
            W0f[:, m, n, :] = 0.0
    W0f = W0f.reshape(TWO_N, K_IN, H)
    shared = {
        "W0": np.ascontiguousarray(W0f),
        "W1": np.ascontiguousarray(np.asarray(W1, np.float32)),
        "W2": np.ascontiguousarray(np.asarray(W2, np.float32)),
        "W3": np.ascontiguousarray(np.asarray(W3, np.float32)),
        "b0": np.ascontiguousarray(np.asarray(b0, np.float32)),
        "b1": np.ascontiguousarray(np.asarray(b1, np.float32)),
        "b2": np.ascontiguousarray(np.asarray(b2, np.float32)),
        "b3": np.ascontiguousarray(np.asarray(b3, np.float32)),
    }
    in_maps = []
    for c in range(N_CORES):
        m = dict(shared)
        m["x"] = x[B_LOC * c:B_LOC * (c + 1), :]
        in_maps.append(m)
    res = run_bass_kernel_spmd(_NC, in_maps, list(range(N_CORES)))
    return np.concatenate([res.results[c]["y"] for c in range(N_CORES)],
                          axis=0)
